# revision 2
# baseline (speedup 1.0000x reference)
"""Trainium2 Bass kernel for nn_ContextGenerator (scatter_memory).

Data-parallel over the batch axis: 64 batches -> 8 NeuronCores x 8 batches.
All weights (conv vectors, attention projections, [M,M] mixing kernel) are
replicated. No collectives.

Internal memory-row order is [context rows 0..1023 ; conv rows 1024..1037]
(the reference order is [conv ; context]); the permutation is folded into the
host-side prep of the mixing kernel and into the output DMA offsets.
"""

import numpy as np

import concourse.bass as bass
import concourse.mybir as mybir
import concourse.tile as tile
from concourse import bacc
from concourse.bass_utils import run_bass_kernel_spmd
from concourse.masks import make_identity

# Problem constants (hardcoded per harness contract)
B = 64
T = 1024
DS, DA, DR = 256, 96, 32
F = DS + DA + DR            # 384
U = 256                     # attention units
NCONV = 14                  # 8 + 4 + 2 compressed slots
M = T + NCONV               # 1038
N_CORES = 8
BL = B // N_CORES           # 8 batches per core
SCALE = 1.0 / 16.0          # 1/sqrt(U)

F32 = mybir.dt.float32
F32R = mybir.dt.float32r

# m-axis tiles: 8 full 128-partition tiles + one 14-row tile
JT = [(j, 128 if j < 8 else NCONV) for j in range(9)]
# free-axis chunks of the m/n axis (psum bank = 512 fp32)
MCH = [(0, 512), (512, 512), (1024, NCONV)]


def r32(ap):
    return ap.bitcast(F32R)


def build_program(n_batch=BL):
    """Build the per-core Bass program. Same program on all 8 cores (SPMD)."""
    nc = bacc.Bacc(None, target_bir_lowering=False)

    xs = nc.declare_dram_parameter("xs", [n_batch, T, DS], F32, isOutput=False)
    xa = nc.declare_dram_parameter("xa", [n_batch, T, DA], F32, isOutput=False)
    xr = nc.declare_dram_parameter("xr", [n_batch, T, DR], F32, isOutput=False)
    ct = nc.declare_dram_parameter("ct", [T, NCONV], F32, isOutput=False)
    wq = nc.declare_dram_parameter("wq", [F, U], F32, isOutput=False)
    bq = nc.declare_dram_parameter("bq", [U], F32, isOutput=False)
    wk = nc.declare_dram_parameter("wk", [F, U], F32, isOutput=False)
    bk = nc.declare_dram_parameter("bk", [U], F32, isOutput=False)
    kt = nc.declare_dram_parameter("kt", [M, M], F32, isOutput=False)
    out = nc.declare_dram_parameter("out", [n_batch, M, F], F32, isOutput=True)
    probs = nc.declare_dram_parameter("probs", [n_batch, M, M], F32, isOutput=True)

    with tile.TileContext(nc) as tc:
        with (
            tc.tile_pool(name="consts", bufs=1) as consts,
            tc.tile_pool(name="mem", bufs=2) as memp,
            tc.tile_pool(name="memT", bufs=1) as memtp,
            tc.tile_pool(name="qk", bufs=1) as qkp,
            tc.tile_pool(name="ET", bufs=1) as etp,
            tc.tile_pool(name="Es", bufs=2) as esp,
            tc.tile_pool(name="att", bufs=2) as attp,
            tc.tile_pool(name="small", bufs=2) as smallp,
            tc.tile_pool(name="obuf", bufs=3) as obufp,
            tc.tile_pool(name="ps_tp", bufs=2, space="PSUM") as ps_tp,
            tc.tile_pool(name="ps_mm", bufs=3, space="PSUM") as ps_mm,
            tc.tile_pool(name="ps_acc", bufs=2, space="PSUM") as ps_acc,
            tc.tile_pool(name="ps_small", bufs=1, space="PSUM") as ps_small,
        ):
            # ---- constants ----
            ident = consts.tile([128, 128], F32, tag="ident")
            make_identity(nc, ident[:])
            ct_sb = consts.tile([128, 8, NCONV], F32, tag="ct")
            nc.sync.dma_start(out=r32(ct_sb[:]), in_=r32(ct.rearrange("(j p) k -> p j k", p=128)))
            wq_sb = consts.tile([128, 3, U], F32, tag="wq")
            nc.sync.dma_start(out=r32(wq_sb[:]), in_=r32(wq.rearrange("(t p) u -> p t u", p=128)))
            wk_sb = consts.tile([128, 3, U], F32, tag="wk")
            nc.sync.dma_start(out=r32(wk_sb[:]), in_=r32(wk.rearrange("(t p) u -> p t u", p=128)))
            bq_sb = consts.tile([128, 2], F32, tag="bq")
            nc.sync.dma_start(out=bq_sb[:], in_=bq.rearrange("(t p) -> p t", p=128))
            bk_sb = consts.tile([128, 2], F32, tag="bk")
            nc.sync.dma_start(out=bk_sb[:], in_=bk.rearrange("(t p) -> p t", p=128))
            kt_sb = consts.tile([128, 9, M], F32, tag="kt")
            nc.sync.dma_start(
                out=r32(kt_sb[:, 0:8, :]),
                in_=r32(kt[0:1024, :].rearrange("(j p) i -> p j i", p=128)),
            )
            nc.sync.dma_start(out=r32(kt_sb[:NCONV, 8, :]), in_=r32(kt[1024:M, :]))

            for b in range(n_batch):
                # ---- load context: mem rows 0..1023 = ctx, 1024..1037 = conv ----
                mem_t = memp.tile([128, 9, F], F32, tag="mem")
                nc.sync.dma_start(
                    out=r32(mem_t[:, 0:8, 0:DS]),
                    in_=r32(xs[b].rearrange("(j p) f -> p j f", p=128)),
                )
                nc.sync.dma_start(
                    out=r32(mem_t[:, 0:8, DS:DS + DA]),
                    in_=r32(xa[b].rearrange("(j p) f -> p j f", p=128)),
                )
                nc.sync.dma_start(
                    out=r32(mem_t[:, 0:8, DS + DA:F]),
                    in_=r32(xr[b].rearrange("(j p) f -> p j f", p=128)),
                )

                # ---- conv rows: [14, F] = C @ ctx ----
                conv_ps = ps_small.tile([NCONV, F], F32, tag="conv")
                for j in range(8):
                    nc.tensor.matmul(
                        conv_ps[:],
                        lhsT=r32(ct_sb[:, j, :]),
                        rhs=r32(mem_t[:, j, :]),
                        start=(j == 0),
                        stop=(j == 7),
                    )
                nc.scalar.copy(out=r32(mem_t[:NCONV, 8, :]), in_=conv_ps[:])

                # ---- memT [f 3x128, m 1038] via PE transposes ----
                memT_t = memtp.tile([128, 3, M], F32, tag="memT")
                for j, pp in JT:
                    for ft in range(3):
                        tp_ps = ps_tp.tile([128, 128], F32, tag="tp")
                        nc.tensor.transpose(
                            tp_ps[:, :pp],
                            in_=mem_t[:pp, j, ft * 128:(ft + 1) * 128],
                            identity=ident[:pp, :pp],
                        )
                        nc.any.tensor_copy(
                            r32(memT_t[:, ft, j * 128:j * 128 + pp]), tp_ps[:, :pp]
                        )

                # ---- qT, kT [u 2x128, m 1038] ----
                qT = qkp.tile([128, 2, M], F32, tag="qT")
                kT = qkp.tile([128, 2, M], F32, tag="kT")
                for wsb, bsb, dst in ((wq_sb, bq_sb, qT), (wk_sb, bk_sb, kT)):
                    for ut in range(2):
                        for m0, mw in MCH:
                            ps = ps_mm.tile([128, 512], F32, tag="mm")
                            for ft in range(3):
                                nc.tensor.matmul(
                                    ps[:, :mw],
                                    lhsT=r32(wsb[:, ft, ut * 128:(ut + 1) * 128]),
                                    rhs=r32(memT_t[:, ft, m0:m0 + mw]),
                                    start=(ft == 0),
                                    stop=(ft == 2),
                                )
                            nc.scalar.activation(
                                out=r32(dst[:, ut, m0:m0 + mw]),
                                in_=ps[:, :mw],
                                func=mybir.ActivationFunctionType.Identity,
                                bias=bsb[:, ut:ut + 1],
                            )

                # ---- E^T = exp(S^T/16), S^T[n,m] ----
                ET_t = etp.tile([128, 9, M], F32, tag="ET")
                for jn, pn in JT:
                    for m0, mw in MCH:
                        ps = ps_mm.tile([128, 512], F32, tag="mm")
                        for ut in range(2):
                            nc.tensor.matmul(
                                ps[:pn, :mw],
                                lhsT=r32(kT[:, ut, jn * 128:jn * 128 + pn]),
                                rhs=r32(qT[:, ut, m0:m0 + mw]),
                                start=(ut == 0),
                                stop=(ut == 1),
                            )
                        nc.scalar.activation(
                            out=r32(ET_t[:pn, jn, m0:m0 + mw]),
                            in_=ps[:pn, :mw],
                            func=mybir.ActivationFunctionType.Exp,
                            scale=SCALE,
                        )

                # ---- S rows -> probs strips + row sums ----
                r_sb = smallp.tile([128, 9], F32, tag="r")
                for jm, pm in JT:
                    es_t = esp.tile([128, M], F32, tag="Es")
                    zsum = smallp.tile([128, 4], F32, tag="z")
                    for ci, (n0, nw) in enumerate(MCH):
                        ps = ps_mm.tile([128, 512], F32, tag="mm")
                        for ut in range(2):
                            nc.tensor.matmul(
                                ps[:pm, :nw],
                                lhsT=r32(qT[:, ut, jm * 128:jm * 128 + pm]),
                                rhs=r32(kT[:, ut, n0:n0 + nw]),
                                start=(ut == 0),
                                stop=(ut == 1),
                            )
                        nc.scalar.activation(
                            out=es_t[:pm, n0:n0 + nw],
                            in_=ps[:pm, :nw],
                            func=mybir.ActivationFunctionType.Exp,
                            scale=SCALE,
                            accum_out=zsum[:pm, ci:ci + 1],
                        )
                    ztot = smallp.tile([128, 1], F32, tag="ztot")
                    nc.vector.reduce_sum(
                        out=ztot[:pm, :], in_=zsum[:pm, 0:3], axis=mybir.AxisListType.X
                    )
                    nc.vector.reciprocal(r_sb[:pm, jm:jm + 1], ztot[:pm, :])
                    nc.vector.tensor_scalar_mul(
                        es_t[:pm, :], es_t[:pm, :], r_sb[:pm, jm:jm + 1]
                    )
                    # DMA probs strip (internal cols [ctx;conv] -> ref [conv;ctx])
                    rr = 14 + jm * 128 if jm < 8 else 0
                    nc.sync.dma_start(
                        out=probs[b, rr:rr + pm, NCONV:M], in_=es_t[:pm, 0:1024]
                    )
                    nc.sync.dma_start(
                        out=probs[b, rr:rr + pm, 0:NCONV], in_=es_t[:pm, 1024:M]
                    )

                # ---- att[m,f] = (1/Z_m) * sum_n E^T[n,m] mem[n,f] ----
                att_t = attp.tile([128, 9, F], F32, tag="att")
                for jm, pm in JT:
                    aps = ps_acc.tile([128, F], F32, tag="acc")
                    for jn, pn in JT:
                        nc.tensor.matmul(
                            aps[:pm, :],
                            lhsT=r32(ET_t[:pn, jn, jm * 128:jm * 128 + pm]),
                            rhs=r32(mem_t[:pn, jn, :]),
                            start=(jn == 0),
                            stop=(jn == 8),
                        )
                    nc.vector.tensor_scalar_mul(
                        r32(att_t[:pm, jm, :]), aps[:pm, :], r_sb[:pm, jm:jm + 1]
                    )

                # ---- out rows: kernel mixing ----
                for ji, pi in JT:
                    fps = ps_acc.tile([128, F], F32, tag="acc")
                    for jm, pm in JT:
                        nc.tensor.matmul(
                            fps[:pi, :],
                            lhsT=r32(kt_sb[:pm, jm, ji * 128:ji * 128 + pi]),
                            rhs=r32(att_t[:pm, jm, :]),
                            start=(jm == 0),
                            stop=(jm == 8),
                        )
                    ob = obufp.tile([128, F], F32, tag="ob")
                    nc.any.tensor_copy(ob[:pi, :], fps[:pi, :])
                    rr = 14 + ji * 128 if ji < 8 else 0
                    nc.sync.dma_start(out=out[b, rr:rr + pi, :], in_=ob[:pi, :])

    nc.compile()
    return nc


def _prep_weights(w1, w2, w3, kernel):
    """Host-side prep of replicated weights (conv matrix C^T and permuted
    transposed mixing kernel)."""
    C = np.zeros((NCONV, T), dtype=np.float32)
    for k in range(8):
        C[k, 128 * k:128 * (k + 1)] = w3
    for j in range(4):
        C[8 + j, 256 * j:256 * (j + 1)] = w2
    for j in range(2):
        C[12 + j, 512 * j:512 * (j + 1)] = w1
    ct = np.ascontiguousarray(C.T)
    # internal index m' -> reference index
    perm = np.concatenate([np.arange(NCONV, M), np.arange(NCONV)])
    kp = np.asarray(kernel, dtype=np.float32)[np.ix_(perm, perm)]
    kt = np.ascontiguousarray(kp.T)
    return ct, kt


_NC_CACHE = {}


def kernel(x_state, x_action, x_reward, w1, w2, w3, Wq, bq, Wk, bk, kernel):
    x_state = np.ascontiguousarray(np.asarray(x_state, dtype=np.float32))
    x_action = np.ascontiguousarray(np.asarray(x_action, dtype=np.float32))
    x_reward = np.ascontiguousarray(np.asarray(x_reward, dtype=np.float32))
    w1 = np.asarray(w1, dtype=np.float32)
    w2 = np.asarray(w2, dtype=np.float32)
    w3 = np.asarray(w3, dtype=np.float32)
    Wq = np.ascontiguousarray(np.asarray(Wq, dtype=np.float32))
    Wk = np.ascontiguousarray(np.asarray(Wk, dtype=np.float32))
    bq = np.ascontiguousarray(np.asarray(bq, dtype=np.float32))
    bk = np.ascontiguousarray(np.asarray(bk, dtype=np.float32))

    ct, kt = _prep_weights(w1, w2, w3, kernel)

    if "nc" not in _NC_CACHE:
        _NC_CACHE["nc"] = build_program(BL)
    nc = _NC_CACHE["nc"]

    in_maps = []
    for c in range(N_CORES):
        sl = slice(c * BL, (c + 1) * BL)
        in_maps.append(
            {
                "xs": x_state[sl],
                "xa": x_action[sl],
                "xr": x_reward[sl],
                "ct": ct,
                "wq": Wq,
                "bq": bq,
                "wk": Wk,
                "bk": bk,
                "kt": kt,
            }
        )

    res = run_bass_kernel_spmd(nc, in_maps, list(range(N_CORES)))
    out = np.concatenate([res.results[c]["out"] for c in range(N_CORES)], axis=0)
    probs = np.concatenate([res.results[c]["probs"] for c in range(N_CORES)], axis=0)
    return out, probs


# revision 28
# speedup vs baseline: 1.2254x; 1.2254x over previous
"""Trainium2 Bass kernel for nn_ContextGenerator (scatter_memory).

Data-parallel over the batch axis: 64 batches -> 8 NeuronCores x 8 batches.
All weights (conv vectors, attention projections, [M,M] mixing kernel) are
replicated. No collectives.

Internal memory-row order is [context rows 0..1023 ; conv rows 1024..1037]
(the reference order is [conv ; context]); the permutation is folded into the
host-side prep of the mixing kernel and into the output DMA offsets.
"""

import numpy as np

import concourse.bass as bass
import concourse.mybir as mybir
import concourse.tile as tile
from concourse import bacc
from concourse.bass_utils import run_bass_kernel_spmd

# Problem constants (hardcoded per harness contract)
B = 64
T = 1024
DS, DA, DR = 256, 96, 32
F = DS + DA + DR            # 384
U = 256                     # attention units
NCONV = 14                  # 8 + 4 + 2 compressed slots
M = T + NCONV               # 1038
N_CORES = 8
BL = B // N_CORES           # 8 batches per core
SCALE = 1.0 / 16.0          # 1/sqrt(U)

F32 = mybir.dt.float32
F32R = mybir.dt.float32r

# m-axis tiles: 8 full 128-partition tiles + one 14-row tile
JT = [(j, 128 if j < 8 else NCONV) for j in range(9)]
# free-axis chunks of the m/n axis (psum bank = 512 fp32)
MCH = [(0, 512), (512, 512), (1024, NCONV)]


def r32(ap):
    return ap.bitcast(F32R)


def build_program(n_batch=BL, repeat=0):
    """Build the per-core Bass program. Same program on all 8 cores (SPMD)."""
    nc = bacc.Bacc(None, target_bir_lowering=False)

    xs = nc.declare_dram_parameter("xs", [n_batch, T, DS], F32, isOutput=False)
    xa = nc.declare_dram_parameter("xa", [n_batch, T, DA], F32, isOutput=False)
    xr = nc.declare_dram_parameter("xr", [n_batch, T, DR], F32, isOutput=False)
    ct = nc.declare_dram_parameter("ct", [T, NCONV], F32, isOutput=False)
    wq = nc.declare_dram_parameter("wq", [F, U], F32, isOutput=False)
    bq = nc.declare_dram_parameter("bq", [U], F32, isOutput=False)
    wk = nc.declare_dram_parameter("wk", [F, U], F32, isOutput=False)
    bk = nc.declare_dram_parameter("bk", [U], F32, isOutput=False)
    kt = nc.declare_dram_parameter("kt", [M, M], F32, isOutput=False)
    out = nc.declare_dram_parameter("out", [n_batch, M, F], F32, isOutput=True)
    probs = nc.declare_dram_parameter("probs", [n_batch, M, M], F32, isOutput=True)

    with tile.TileContext(nc) as tc:
        with (
            tc.tile_pool(name="consts", bufs=1) as consts,
            tc.tile_pool(name="mem", bufs=3) as memp,
            tc.tile_pool(name="memT", bufs=1) as memtp,
            tc.tile_pool(name="qk", bufs=1) as qkp,
            tc.tile_pool(name="ET", bufs=1) as etp,
            tc.tile_pool(name="Es", bufs=4) as esp,
            tc.tile_pool(name="att", bufs=2) as attp,
            tc.tile_pool(name="small", bufs=2) as smallp,
            tc.tile_pool(name="obuf", bufs=4) as obufp,
            tc.tile_pool(name="ps_tp", bufs=2, space="PSUM") as ps_tp,
            tc.tile_pool(name="ps_mm", bufs=4, space="PSUM") as ps_mm,
            tc.tile_pool(name="ps_acc", bufs=2, space="PSUM") as ps_acc,
        ):
            # ---- constants ----
            ident = consts.tile([128, 128], F32, tag="ident")
            nc.gpsimd.memset(ident[:], 0.0)
            nc.gpsimd.affine_select(
                out=ident[:], in_=ident[:],
                compare_op=mybir.AluOpType.not_equal, fill=1.0, base=0,
                pattern=[[-1, 128]], channel_multiplier=1,
            )
            # rounded copy for the f32r transposes (0.0/1.0 are exact in
            # any rounding); a separate tile keeps the f32 producers out of
            # the verifier's f32r producer chain
            identr = consts.tile([128, 128], F32, tag="identr")
            nc.vector.tensor_copy(r32(identr[:]), ident[:])
            ct_sb = consts.tile([128, 8, NCONV], F32, tag="ct")
            nc.sync.dma_start(out=r32(ct_sb[:]), in_=r32(ct.rearrange("(j p) k -> p j k", p=128)))
            wq_sb = consts.tile([128, 3, U], F32, tag="wq")
            wk_sb = consts.tile([128, 3, U], F32, tag="wk")
            bq_sb = consts.tile([128, 2], F32, tag="bq")
            bk_sb = consts.tile([128, 2], F32, tag="bk")
            kt_sb = consts.tile([128, 9, M], F32, tag="kt")

            import contextlib
            loop_cm = tc.For_i(0, repeat, 1) if repeat else contextlib.nullcontext()
            with loop_cm:
              for b in range(n_batch):
                  # ---- load context: mem rows 0..1023 = ctx, 1024..1037 = conv ----
                  mem_t = memp.tile([128, 9, F], F32, tag="mem")
                  xsr = xs[b].rearrange("(j p) f -> p j f", p=128)
                  xar = xa[b].rearrange("(j p) f -> p j f", p=128)
                  xrr = xr[b].rearrange("(j p) f -> p j f", p=128)
                  # two j-groups per tensor: conv/transposes for tiles 0..3
                  # can start as soon as the first half arrives
                  for g0, g1 in ((0, 4), (4, 8)):
                      nc.sync.dma_start(
                          out=r32(mem_t[:, g0:g1, 0:DS]),
                          in_=r32(xsr[:, g0:g1, :]),
                      )
                      nc.sync.dma_start(
                          out=r32(mem_t[:, g0:g1, DS:DS + DA]),
                          in_=r32(xar[:, g0:g1, :]),
                      )
                      nc.sync.dma_start(
                          out=r32(mem_t[:, g0:g1, DS + DA:F]),
                          in_=r32(xrr[:, g0:g1, :]),
                      )

                  if b == 0:
                      # projection weights are first needed by qkT; emitting
                      # them after the ctx DMAs keeps the conv start unblocked
                      nc.sync.dma_start(
                          out=r32(wq_sb[:]),
                          in_=r32(wq.rearrange("(t p) u -> p t u", p=128)),
                      )
                      nc.sync.dma_start(
                          out=r32(wk_sb[:]),
                          in_=r32(wk.rearrange("(t p) u -> p t u", p=128)),
                      )
                      nc.sync.dma_start(
                          out=bq_sb[:], in_=bq.rearrange("(t p) -> p t", p=128)
                      )
                      nc.sync.dma_start(
                          out=bk_sb[:], in_=bk.rearrange("(t p) -> p t", p=128)
                      )

                  # ---- conv rows: [14, F] = C @ ctx ----
                  conv_ps = ps_tp.tile([128, 512], F32, tag="tp")
                  for j in range(8):
                      nc.tensor.matmul(
                          conv_ps[:NCONV, :F],
                          lhsT=r32(ct_sb[:, j, :]),
                          rhs=r32(mem_t[:, j, :]),
                          start=(j == 0),
                          stop=(j == 7),
                      )
                  nc.scalar.copy(out=r32(mem_t[:NCONV, 8, :]), in_=conv_ps[:NCONV, :F])

                  # ---- memT via PE transposes ----
                  # 4 transposes share one [128,512] psum bank; one wide
                  # psum->sbuf copy per group keeps the copy count low, and the
                  # qkT matmuls for chunk c are emitted right after the groups
                  # they need so PE always has matmul work while copies drain
                  memT_t = memtp.tile([128, 3, M], F32, tag="memT")
                  qT = qkp.tile([128, 2, M], F32, tag="qT")
                  kT = qkp.tile([128, 2, M], F32, tag="kT")
                  JG = [(0, [0, 1, 2, 3]), (1, [4, 5, 6, 7]), (2, [8])]
                  for ci, js in JG:
                      m0, mw = MCH[ci]
                      for ft in range(3):
                          tg_ps = ps_tp.tile([128, 512], F32, tag="tp")
                          for gi, j in enumerate(js):
                              pp = JT[j][1]
                              nc.tensor.transpose(
                                  r32(tg_ps[:, gi * 128:gi * 128 + pp]),
                                  in_=r32(mem_t[:pp, j, ft * 128:(ft + 1) * 128]),
                                  identity=r32(identr[:pp, :pp]),
                              )
                          if ft % 2 == 0:
                              nc.vector.tensor_copy(
                                  r32(memT_t[:, ft, m0:m0 + mw]), tg_ps[:, :mw]
                              )
                          else:
                              nc.scalar.copy(
                                  out=r32(memT_t[:, ft, m0:m0 + mw]),
                                  in_=tg_ps[:, :mw],
                              )
                      for wsb, bsb, dst in ((wq_sb, bq_sb, qT), (wk_sb, bk_sb, kT)):
                          for ut in range(2):
                              ps = ps_mm.tile([128, 512], F32, tag="mm")
                              for ft in range(3):
                                  nc.tensor.matmul(
                                      ps[:, :mw],
                                      lhsT=r32(wsb[:, ft, ut * 128:(ut + 1) * 128]),
                                      rhs=r32(memT_t[:, ft, m0:m0 + mw]),
                                      start=(ft == 0),
                                      stop=(ft == 2),
                                  )
                              nc.scalar.activation(
                                  out=r32(dst[:, ut, m0:m0 + mw]),
                                  in_=ps[:, :mw],
                                  func=mybir.ActivationFunctionType.Identity,
                                  bias=bsb[:, ut:ut + 1],
                              )

                  if b == 0:
                      # the mixing kernel is first needed by the kernel-mixing
                      # phase; loading it here keeps batch-0 ctx DMAs unblocked
                      ktr = kt[0:1024, :].rearrange("(j p) i -> p j i", p=128)
                      nc.sync.dma_start(
                          out=r32(kt_sb[:, 0:3, :]), in_=r32(ktr[:, 0:3, :])
                      )
                      nc.sync.dma_start(
                          out=r32(kt_sb[:, 3:6, :]), in_=r32(ktr[:, 3:6, :])
                      )
                      nc.sync.dma_start(
                          out=r32(kt_sb[:, 6:8, :]), in_=r32(ktr[:, 6:8, :])
                      )
                      nc.sync.dma_start(
                          out=r32(kt_sb[:NCONV, 8, :]), in_=r32(kt[1024:M, :])
                      )

                  # ---- S rows -> probs strips + E^T chunks + att ----
                  # E^T m-chunk c is emitted right before the strips whose
                  # att contraction needs it (jm 0..3 -> chunk0, 4..7 ->
                  # chunk1, 8 -> chunk2), spreading the ACT exp load
                  ET_t = etp.tile([128, 9, M], F32, tag="ET")
                  att_t = attp.tile([128, 9, F], F32, tag="att")
                  r_sb = smallp.tile([128, 9], F32, tag="r")
                  for jm, pm in JT:
                      if jm in (0, 4, 8):
                          m0, mw = MCH[{0: 0, 4: 1, 8: 2}[jm]]
                          for jn, pn in JT:
                              ps = ps_mm.tile([128, 512], F32, tag="mm")
                              for ut in range(2):
                                  nc.tensor.matmul(
                                      ps[:pn, :mw],
                                      lhsT=r32(kT[:, ut, jn * 128:jn * 128 + pn]),
                                      rhs=r32(qT[:, ut, m0:m0 + mw]),
                                      start=(ut == 0),
                                      stop=(ut == 1),
                                  )
                              nc.scalar.activation(
                                  out=r32(ET_t[:pn, jn, m0:m0 + mw]),
                                  in_=ps[:pn, :mw],
                                  func=mybir.ActivationFunctionType.Exp,
                                  scale=SCALE,
                              )
                      es_t = esp.tile([128, M], F32, tag="Es")
                      zsum = smallp.tile([128, 4], F32, tag="z")
                      for ci, (n0, nw) in enumerate(MCH):
                          ps = ps_mm.tile([128, 512], F32, tag="mm")
                          for ut in range(2):
                              nc.tensor.matmul(
                                  ps[:pm, :nw],
                                  lhsT=r32(qT[:, ut, jm * 128:jm * 128 + pm]),
                                  rhs=r32(kT[:, ut, n0:n0 + nw]),
                                  start=(ut == 0),
                                  stop=(ut == 1),
                              )
                          nc.scalar.activation(
                              out=es_t[:pm, n0:n0 + nw],
                              in_=ps[:pm, :nw],
                              func=mybir.ActivationFunctionType.Exp,
                              scale=SCALE,
                              accum_out=zsum[:pm, ci:ci + 1],
                          )
                      ztot = smallp.tile([128, 1], F32, tag="ztot")
                      nc.vector.reduce_sum(
                          out=ztot[:pm, :], in_=zsum[:pm, 0:3], axis=mybir.AxisListType.X
                      )
                      nc.vector.reciprocal(r_sb[:pm, jm:jm + 1], ztot[:pm, :])
                      nc.vector.tensor_scalar_mul(
                          es_t[:pm, :], es_t[:pm, :], r_sb[:pm, jm:jm + 1]
                      )
                      # DMA probs strip (internal cols [ctx;conv] -> ref [conv;ctx])
                      rr = 14 + jm * 128 if jm < 8 else 0
                      nc.sync.dma_start(
                          out=probs[b, rr:rr + pm, NCONV:M], in_=es_t[:pm, 0:1024]
                      )
                      nc.sync.dma_start(
                          out=probs[b, rr:rr + pm, 0:NCONV], in_=es_t[:pm, 1024:M]
                      )

                      # att(jm) interleaved into the strip loop: balances the
                      # ACT exp load across the S window instead of bunching
                      # all att matmuls after it
                      aps = ps_acc.tile([128, F], F32, tag="acc")
                      for jn, pn in JT:
                          nc.tensor.matmul(
                              aps[:pm, :],
                              lhsT=r32(ET_t[:pn, jn, jm * 128:jm * 128 + pm]),
                              rhs=r32(mem_t[:pn, jn, :]),
                              start=(jn == 0),
                              stop=(jn == 8),
                          )
                      nc.vector.tensor_scalar_mul(
                          r32(att_t[:pm, jm, :]), aps[:pm, :], r_sb[:pm, jm:jm + 1]
                      )

                  # ---- out rows: kernel mixing ----
                  for ji, pi in JT:
                      fps = ps_acc.tile([128, F], F32, tag="acc")
                      for jm, pm in JT:
                          nc.tensor.matmul(
                              fps[:pi, :],
                              lhsT=r32(kt_sb[:pm, jm, ji * 128:ji * 128 + pi]),
                              rhs=r32(att_t[:pm, jm, :]),
                              start=(jm == 0),
                              stop=(jm == 8),
                          )
                      ob = obufp.tile([128, F], F32, tag="ob")
                      nc.vector.tensor_copy(ob[:pi, :], fps[:pi, :])
                      rr = 14 + ji * 128 if ji < 8 else 0
                      nc.sync.dma_start(out=out[b, rr:rr + pi, :], in_=ob[:pi, :])

    nc.compile()
    return nc


def _prep_weights(w1, w2, w3, kernel):
    """Host-side prep of replicated weights (conv matrix C^T and permuted
    transposed mixing kernel)."""
    C = np.zeros((NCONV, T), dtype=np.float32)
    for k in range(8):
        C[k, 128 * k:128 * (k + 1)] = w3
    for j in range(4):
        C[8 + j, 256 * j:256 * (j + 1)] = w2
    for j in range(2):
        C[12 + j, 512 * j:512 * (j + 1)] = w1
    ct = np.ascontiguousarray(C.T)
    # internal index m' -> reference index
    perm = np.concatenate([np.arange(NCONV, M), np.arange(NCONV)])
    kp = np.asarray(kernel, dtype=np.float32)[np.ix_(perm, perm)]
    kt = np.ascontiguousarray(kp.T)
    return ct, kt


_NC_CACHE = {}


def kernel(x_state, x_action, x_reward, w1, w2, w3, Wq, bq, Wk, bk, kernel):
    x_state = np.ascontiguousarray(np.asarray(x_state, dtype=np.float32))
    x_action = np.ascontiguousarray(np.asarray(x_action, dtype=np.float32))
    x_reward = np.ascontiguousarray(np.asarray(x_reward, dtype=np.float32))
    w1 = np.asarray(w1, dtype=np.float32)
    w2 = np.asarray(w2, dtype=np.float32)
    w3 = np.asarray(w3, dtype=np.float32)
    Wq = np.ascontiguousarray(np.asarray(Wq, dtype=np.float32))
    Wk = np.ascontiguousarray(np.asarray(Wk, dtype=np.float32))
    bq = np.ascontiguousarray(np.asarray(bq, dtype=np.float32))
    bk = np.ascontiguousarray(np.asarray(bk, dtype=np.float32))

    ct, kt = _prep_weights(w1, w2, w3, kernel)

    if "nc" not in _NC_CACHE:
        _NC_CACHE["nc"] = build_program(BL)
    nc = _NC_CACHE["nc"]

    in_maps = []
    for c in range(N_CORES):
        sl = slice(c * BL, (c + 1) * BL)
        in_maps.append(
            {
                "xs": x_state[sl],
                "xa": x_action[sl],
                "xr": x_reward[sl],
                "ct": ct,
                "wq": Wq,
                "bq": bq,
                "wk": Wk,
                "bk": bk,
                "kt": kt,
            }
        )

    res = run_bass_kernel_spmd(nc, in_maps, list(range(N_CORES)))
    out = np.concatenate([res.results[c]["out"] for c in range(N_CORES)], axis=0)
    probs = np.concatenate([res.results[c]["probs"] for c in range(N_CORES)], axis=0)
    return out, probs



# revision 33
# speedup vs baseline: 1.2260x; 1.0005x over previous
"""Trainium2 Bass kernel for nn_ContextGenerator (scatter_memory).

Data-parallel over the batch axis: 64 batches -> 8 NeuronCores x 8 batches.
All weights (conv vectors, attention projections, [M,M] mixing kernel) are
replicated. No collectives.

Internal memory-row order is [context rows 0..1023 ; conv rows 1024..1037]
(the reference order is [conv ; context]); the permutation is folded into the
host-side prep of the mixing kernel and into the output DMA offsets.
"""

import numpy as np

import concourse.bass as bass
import concourse.mybir as mybir
import concourse.tile as tile
from concourse import bacc
from concourse.bass_utils import run_bass_kernel_spmd

# Problem constants (hardcoded per harness contract)
B = 64
T = 1024
DS, DA, DR = 256, 96, 32
F = DS + DA + DR            # 384
U = 256                     # attention units
NCONV = 14                  # 8 + 4 + 2 compressed slots
M = T + NCONV               # 1038
N_CORES = 8
BL = B // N_CORES           # 8 batches per core
SCALE = 1.0 / 16.0          # 1/sqrt(U)

F32 = mybir.dt.float32
F32R = mybir.dt.float32r

# m-axis tiles: 8 full 128-partition tiles + one 14-row tile
JT = [(j, 128 if j < 8 else NCONV) for j in range(9)]
# free-axis chunks of the m/n axis (psum bank = 512 fp32)
MCH = [(0, 512), (512, 512), (1024, NCONV)]


def r32(ap):
    return ap.bitcast(F32R)


def build_program(n_batch=BL, repeat=0):
    """Build the per-core Bass program. Same program on all 8 cores (SPMD)."""
    nc = bacc.Bacc(None, target_bir_lowering=False)

    xs = nc.declare_dram_parameter("xs", [n_batch, T, DS], F32, isOutput=False)
    xa = nc.declare_dram_parameter("xa", [n_batch, T, DA], F32, isOutput=False)
    xr = nc.declare_dram_parameter("xr", [n_batch, T, DR], F32, isOutput=False)
    ct = nc.declare_dram_parameter("ct", [T, NCONV], F32, isOutput=False)
    wq = nc.declare_dram_parameter("wq", [F, U], F32, isOutput=False)
    bq = nc.declare_dram_parameter("bq", [U], F32, isOutput=False)
    wk = nc.declare_dram_parameter("wk", [F, U], F32, isOutput=False)
    bk = nc.declare_dram_parameter("bk", [U], F32, isOutput=False)
    kt = nc.declare_dram_parameter("kt", [M, M], F32, isOutput=False)
    out = nc.declare_dram_parameter("out", [n_batch, M, F], F32, isOutput=True)
    probs = nc.declare_dram_parameter("probs", [n_batch, M, M], F32, isOutput=True)

    with tile.TileContext(nc) as tc:
        with (
            tc.tile_pool(name="consts", bufs=1) as consts,
            tc.tile_pool(name="mem", bufs=3) as memp,
            tc.tile_pool(name="memT", bufs=1) as memtp,
            tc.tile_pool(name="qk", bufs=1) as qkp,
            tc.tile_pool(name="ET", bufs=1) as etp,
            tc.tile_pool(name="Es", bufs=4) as esp,
            tc.tile_pool(name="att", bufs=2) as attp,
            tc.tile_pool(name="small", bufs=2) as smallp,
            tc.tile_pool(name="obuf", bufs=4) as obufp,
            tc.tile_pool(name="ps_tp", bufs=2, space="PSUM") as ps_tp,
            tc.tile_pool(name="ps_mm", bufs=4, space="PSUM") as ps_mm,
            tc.tile_pool(name="ps_acc", bufs=2, space="PSUM") as ps_acc,
        ):
            # ---- constants ----
            ident = consts.tile([128, 128], F32, tag="ident")
            nc.gpsimd.memset(ident[:], 0.0)
            nc.gpsimd.affine_select(
                out=ident[:], in_=ident[:],
                compare_op=mybir.AluOpType.not_equal, fill=1.0, base=0,
                pattern=[[-1, 128]], channel_multiplier=1,
            )
            # rounded copy for the f32r transposes (0.0/1.0 are exact in
            # any rounding); a separate tile keeps the f32 producers out of
            # the verifier's f32r producer chain
            identr = consts.tile([128, 128], F32, tag="identr")
            nc.vector.tensor_copy(r32(identr[:]), ident[:])
            ct_sb = consts.tile([128, 8, NCONV], F32, tag="ct")
            # issue from the ACT sequencer: SP's serial ~1us/DMA descriptor
            # chain for the batch-0 ctx loads stays one entry shorter
            nc.scalar.dma_start(out=r32(ct_sb[:]), in_=r32(ct.rearrange("(j p) k -> p j k", p=128)))
            wq_sb = consts.tile([128, 3, U], F32, tag="wq")
            wk_sb = consts.tile([128, 3, U], F32, tag="wk")
            bq_sb = consts.tile([128, 2], F32, tag="bq")
            bk_sb = consts.tile([128, 2], F32, tag="bk")
            kt_sb = consts.tile([128, 9, M], F32, tag="kt")

            import contextlib
            loop_cm = tc.For_i(0, repeat, 1) if repeat else contextlib.nullcontext()
            with loop_cm:
              for b in range(n_batch):
                  # ---- load context: mem rows 0..1023 = ctx, 1024..1037 = conv ----
                  mem_t = memp.tile([128, 9, F], F32, tag="mem")
                  xsr = xs[b].rearrange("(j p) f -> p j f", p=128)
                  xar = xa[b].rearrange("(j p) f -> p j f", p=128)
                  xrr = xr[b].rearrange("(j p) f -> p j f", p=128)
                  # two j-groups per tensor: conv/transposes for tiles 0..3
                  # can start as soon as the first half arrives
                  for g0, g1 in ((0, 4), (4, 8)):
                      nc.sync.dma_start(
                          out=r32(mem_t[:, g0:g1, 0:DS]),
                          in_=r32(xsr[:, g0:g1, :]),
                      )
                      nc.sync.dma_start(
                          out=r32(mem_t[:, g0:g1, DS:DS + DA]),
                          in_=r32(xar[:, g0:g1, :]),
                      )
                      nc.sync.dma_start(
                          out=r32(mem_t[:, g0:g1, DS + DA:F]),
                          in_=r32(xrr[:, g0:g1, :]),
                      )

                  if b == 0:
                      # projection weights are first needed by qkT; emitting
                      # them after the ctx DMAs keeps the conv start unblocked
                      nc.sync.dma_start(
                          out=r32(wq_sb[:]),
                          in_=r32(wq.rearrange("(t p) u -> p t u", p=128)),
                      )
                      nc.sync.dma_start(
                          out=r32(wk_sb[:]),
                          in_=r32(wk.rearrange("(t p) u -> p t u", p=128)),
                      )
                      nc.sync.dma_start(
                          out=bq_sb[:], in_=bq.rearrange("(t p) -> p t", p=128)
                      )
                      nc.sync.dma_start(
                          out=bk_sb[:], in_=bk.rearrange("(t p) -> p t", p=128)
                      )

                  # ---- conv rows: [14, F] = C @ ctx ----
                  conv_ps = ps_tp.tile([128, 512], F32, tag="tp")
                  for j in range(8):
                      nc.tensor.matmul(
                          conv_ps[:NCONV, :F],
                          lhsT=r32(ct_sb[:, j, :]),
                          rhs=r32(mem_t[:, j, :]),
                          start=(j == 0),
                          stop=(j == 7),
                      )
                  nc.scalar.copy(out=r32(mem_t[:NCONV, 8, :]), in_=conv_ps[:NCONV, :F])

                  # ---- memT via PE transposes ----
                  # 4 transposes share one [128,512] psum bank; one wide
                  # psum->sbuf copy per group keeps the copy count low, and the
                  # qkT matmuls for chunk c are emitted right after the groups
                  # they need so PE always has matmul work while copies drain
                  memT_t = memtp.tile([128, 3, M], F32, tag="memT")
                  qT = qkp.tile([128, 2, M], F32, tag="qT")
                  kT = qkp.tile([128, 2, M], F32, tag="kT")
                  JG = [(0, [0, 1, 2, 3]), (1, [4, 5, 6, 7]), (2, [8])]
                  for ci, js in JG:
                      m0, mw = MCH[ci]
                      for ft in range(3):
                          tg_ps = ps_tp.tile([128, 512], F32, tag="tp")
                          for gi, j in enumerate(js):
                              pp = JT[j][1]
                              nc.tensor.transpose(
                                  r32(tg_ps[:, gi * 128:gi * 128 + pp]),
                                  in_=r32(mem_t[:pp, j, ft * 128:(ft + 1) * 128]),
                                  identity=r32(identr[:pp, :pp]),
                              )
                          if ft % 2 == 0:
                              nc.vector.tensor_copy(
                                  r32(memT_t[:, ft, m0:m0 + mw]), tg_ps[:, :mw]
                              )
                          else:
                              nc.scalar.copy(
                                  out=r32(memT_t[:, ft, m0:m0 + mw]),
                                  in_=tg_ps[:, :mw],
                              )
                      for wsb, bsb, dst in ((wq_sb, bq_sb, qT), (wk_sb, bk_sb, kT)):
                          for ut in range(2):
                              ps = ps_mm.tile([128, 512], F32, tag="mm")
                              for ft in range(3):
                                  nc.tensor.matmul(
                                      ps[:, :mw],
                                      lhsT=r32(wsb[:, ft, ut * 128:(ut + 1) * 128]),
                                      rhs=r32(memT_t[:, ft, m0:m0 + mw]),
                                      start=(ft == 0),
                                      stop=(ft == 2),
                                  )
                              nc.scalar.activation(
                                  out=r32(dst[:, ut, m0:m0 + mw]),
                                  in_=ps[:, :mw],
                                  func=mybir.ActivationFunctionType.Identity,
                                  bias=bsb[:, ut:ut + 1],
                              )

                  if b == 0:
                      # the mixing kernel is first needed by the kernel-mixing
                      # phase; loading it here keeps batch-0 ctx DMAs unblocked
                      ktr = kt[0:1024, :].rearrange("(j p) i -> p j i", p=128)
                      nc.sync.dma_start(
                          out=r32(kt_sb[:, 0:3, :]), in_=r32(ktr[:, 0:3, :])
                      )
                      nc.sync.dma_start(
                          out=r32(kt_sb[:, 3:6, :]), in_=r32(ktr[:, 3:6, :])
                      )
                      nc.sync.dma_start(
                          out=r32(kt_sb[:, 6:8, :]), in_=r32(ktr[:, 6:8, :])
                      )
                      nc.sync.dma_start(
                          out=r32(kt_sb[:NCONV, 8, :]), in_=r32(kt[1024:M, :])
                      )

                  # ---- S rows -> probs strips + E^T chunks + att ----
                  # E^T m-chunk c is emitted right before the strips whose
                  # att contraction needs it (jm 0..3 -> chunk0, 4..7 ->
                  # chunk1, 8 -> chunk2), spreading the ACT exp load
                  ET_t = etp.tile([128, 9, M], F32, tag="ET")
                  att_t = attp.tile([128, 9, F], F32, tag="att")
                  r_sb = smallp.tile([128, 9], F32, tag="r")
                  for jm, pm in JT:
                      if jm in (0, 4, 8):
                          m0, mw = MCH[{0: 0, 4: 1, 8: 2}[jm]]
                          for jn, pn in JT:
                              ps = ps_mm.tile([128, 512], F32, tag="mm")
                              for ut in range(2):
                                  nc.tensor.matmul(
                                      ps[:pn, :mw],
                                      lhsT=r32(kT[:, ut, jn * 128:jn * 128 + pn]),
                                      rhs=r32(qT[:, ut, m0:m0 + mw]),
                                      start=(ut == 0),
                                      stop=(ut == 1),
                                  )
                              nc.scalar.activation(
                                  out=r32(ET_t[:pn, jn, m0:m0 + mw]),
                                  in_=ps[:pn, :mw],
                                  func=mybir.ActivationFunctionType.Exp,
                                  scale=SCALE,
                              )
                      es_t = esp.tile([128, M], F32, tag="Es")
                      zsum = smallp.tile([128, 4], F32, tag="z")
                      for ci, (n0, nw) in enumerate(MCH):
                          ps = ps_mm.tile([128, 512], F32, tag="mm")
                          for ut in range(2):
                              nc.tensor.matmul(
                                  ps[:pm, :nw],
                                  lhsT=r32(qT[:, ut, jm * 128:jm * 128 + pm]),
                                  rhs=r32(kT[:, ut, n0:n0 + nw]),
                                  start=(ut == 0),
                                  stop=(ut == 1),
                              )
                          nc.scalar.activation(
                              out=es_t[:pm, n0:n0 + nw],
                              in_=ps[:pm, :nw],
                              func=mybir.ActivationFunctionType.Exp,
                              scale=SCALE,
                              accum_out=zsum[:pm, ci:ci + 1],
                          )
                      ztot = smallp.tile([128, 1], F32, tag="ztot")
                      nc.vector.reduce_sum(
                          out=ztot[:pm, :], in_=zsum[:pm, 0:3], axis=mybir.AxisListType.X
                      )
                      nc.vector.reciprocal(r_sb[:pm, jm:jm + 1], ztot[:pm, :])
                      nc.vector.tensor_scalar_mul(
                          es_t[:pm, :], es_t[:pm, :], r_sb[:pm, jm:jm + 1]
                      )
                      # DMA probs strip (internal cols [ctx;conv] -> ref [conv;ctx])
                      rr = 14 + jm * 128 if jm < 8 else 0
                      nc.sync.dma_start(
                          out=probs[b, rr:rr + pm, NCONV:M], in_=es_t[:pm, 0:1024]
                      )
                      nc.sync.dma_start(
                          out=probs[b, rr:rr + pm, 0:NCONV], in_=es_t[:pm, 1024:M]
                      )

                      # att(jm) interleaved into the strip loop: balances the
                      # ACT exp load across the S window instead of bunching
                      # all att matmuls after it
                      aps = ps_acc.tile([128, F], F32, tag="acc")
                      for jn, pn in JT:
                          nc.tensor.matmul(
                              aps[:pm, :],
                              lhsT=r32(ET_t[:pn, jn, jm * 128:jm * 128 + pm]),
                              rhs=r32(mem_t[:pn, jn, :]),
                              start=(jn == 0),
                              stop=(jn == 8),
                          )
                      nc.vector.tensor_scalar_mul(
                          r32(att_t[:pm, jm, :]), aps[:pm, :], r_sb[:pm, jm:jm + 1]
                      )

                  # ---- out rows: kernel mixing ----
                  for ji, pi in JT:
                      fps = ps_acc.tile([128, F], F32, tag="acc")
                      for jm, pm in JT:
                          nc.tensor.matmul(
                              fps[:pi, :],
                              lhsT=r32(kt_sb[:pm, jm, ji * 128:ji * 128 + pi]),
                              rhs=r32(att_t[:pm, jm, :]),
                              start=(jm == 0),
                              stop=(jm == 8),
                          )
                      ob = obufp.tile([128, F], F32, tag="ob")
                      nc.vector.tensor_copy(ob[:pi, :], fps[:pi, :])
                      rr = 14 + ji * 128 if ji < 8 else 0
                      nc.sync.dma_start(out=out[b, rr:rr + pi, :], in_=ob[:pi, :])

    nc.compile()
    return nc


def _prep_weights(w1, w2, w3, kernel):
    """Host-side prep of replicated weights (conv matrix C^T and permuted
    transposed mixing kernel)."""
    C = np.zeros((NCONV, T), dtype=np.float32)
    for k in range(8):
        C[k, 128 * k:128 * (k + 1)] = w3
    for j in range(4):
        C[8 + j, 256 * j:256 * (j + 1)] = w2
    for j in range(2):
        C[12 + j, 512 * j:512 * (j + 1)] = w1
    ct = np.ascontiguousarray(C.T)
    # internal index m' -> reference index
    perm = np.concatenate([np.arange(NCONV, M), np.arange(NCONV)])
    kp = np.asarray(kernel, dtype=np.float32)[np.ix_(perm, perm)]
    kt = np.ascontiguousarray(kp.T)
    return ct, kt


_NC_CACHE = {}


def kernel(x_state, x_action, x_reward, w1, w2, w3, Wq, bq, Wk, bk, kernel):
    x_state = np.ascontiguousarray(np.asarray(x_state, dtype=np.float32))
    x_action = np.ascontiguousarray(np.asarray(x_action, dtype=np.float32))
    x_reward = np.ascontiguousarray(np.asarray(x_reward, dtype=np.float32))
    w1 = np.asarray(w1, dtype=np.float32)
    w2 = np.asarray(w2, dtype=np.float32)
    w3 = np.asarray(w3, dtype=np.float32)
    Wq = np.ascontiguousarray(np.asarray(Wq, dtype=np.float32))
    Wk = np.ascontiguousarray(np.asarray(Wk, dtype=np.float32))
    bq = np.ascontiguousarray(np.asarray(bq, dtype=np.float32))
    bk = np.ascontiguousarray(np.asarray(bk, dtype=np.float32))

    ct, kt = _prep_weights(w1, w2, w3, kernel)

    if "nc" not in _NC_CACHE:
        _NC_CACHE["nc"] = build_program(BL)
    nc = _NC_CACHE["nc"]

    in_maps = []
    for c in range(N_CORES):
        sl = slice(c * BL, (c + 1) * BL)
        in_maps.append(
            {
                "xs": x_state[sl],
                "xa": x_action[sl],
                "xr": x_reward[sl],
                "ct": ct,
                "wq": Wq,
                "bq": bq,
                "wk": Wk,
                "bk": bk,
                "kt": kt,
            }
        )

    res = run_bass_kernel_spmd(nc, in_maps, list(range(N_CORES)))
    out = np.concatenate([res.results[c]["out"] for c in range(N_CORES)], axis=0)
    probs = np.concatenate([res.results[c]["probs"] for c in range(N_CORES)], axis=0)
    return out, probs



# revision 34
# speedup vs baseline: 1.2432x; 1.0140x over previous
"""Trainium2 Bass kernel for nn_ContextGenerator (scatter_memory).

Data-parallel over the batch axis: 64 batches -> 8 NeuronCores x 8 batches.
All weights (conv vectors, attention projections, [M,M] mixing kernel) are
replicated. No collectives.

Internal memory-row order is [context rows 0..1023 ; conv rows 1024..1037]
(the reference order is [conv ; context]); the permutation is folded into the
host-side prep of the mixing kernel and into the output DMA offsets.
"""

import numpy as np

import concourse.bass as bass
import concourse.mybir as mybir
import concourse.tile as tile
from concourse import bacc
from concourse.bass_utils import run_bass_kernel_spmd

# Problem constants (hardcoded per harness contract)
B = 64
T = 1024
DS, DA, DR = 256, 96, 32
F = DS + DA + DR            # 384
U = 256                     # attention units
NCONV = 14                  # 8 + 4 + 2 compressed slots
M = T + NCONV               # 1038
N_CORES = 8
BL = B // N_CORES           # 8 batches per core
SCALE = 1.0 / 16.0          # 1/sqrt(U)

F32 = mybir.dt.float32
F32R = mybir.dt.float32r

# m-axis tiles: 8 full 128-partition tiles + one 14-row tile
JT = [(j, 128 if j < 8 else NCONV) for j in range(9)]
# free-axis chunks of the m/n axis (psum bank = 512 fp32)
MCH = [(0, 512), (512, 512), (1024, NCONV)]
# score-phase chunks: every chunk >= 256 wide so f32r streams at 1 cyc/row
# (widths < 256 pay a 4x penalty); aligned to strip groups 0-2 / 3-5 / 6-8
SCH = [(0, 384), (384, 384), (768, 270)]


def r32(ap):
    return ap.bitcast(F32R)


def build_program(n_batch=BL, repeat=0):
    """Build the per-core Bass program. Same program on all 8 cores (SPMD)."""
    nc = bacc.Bacc(None, target_bir_lowering=False)

    xs = nc.declare_dram_parameter("xs", [n_batch, T, DS], F32, isOutput=False)
    xa = nc.declare_dram_parameter("xa", [n_batch, T, DA], F32, isOutput=False)
    xr = nc.declare_dram_parameter("xr", [n_batch, T, DR], F32, isOutput=False)
    ct = nc.declare_dram_parameter("ct", [T, NCONV], F32, isOutput=False)
    wq = nc.declare_dram_parameter("wq", [F, U], F32, isOutput=False)
    bq = nc.declare_dram_parameter("bq", [U], F32, isOutput=False)
    wk = nc.declare_dram_parameter("wk", [F, U], F32, isOutput=False)
    bk = nc.declare_dram_parameter("bk", [U], F32, isOutput=False)
    kt = nc.declare_dram_parameter("kt", [M, M], F32, isOutput=False)
    out = nc.declare_dram_parameter("out", [n_batch, M, F], F32, isOutput=True)
    probs = nc.declare_dram_parameter("probs", [n_batch, M, M], F32, isOutput=True)

    with tile.TileContext(nc) as tc:
        with (
            tc.tile_pool(name="consts", bufs=1) as consts,
            tc.tile_pool(name="mem", bufs=3) as memp,
            tc.tile_pool(name="memT", bufs=1) as memtp,
            tc.tile_pool(name="qk", bufs=1) as qkp,
            tc.tile_pool(name="ET", bufs=1) as etp,
            tc.tile_pool(name="Es", bufs=4) as esp,
            tc.tile_pool(name="att", bufs=2) as attp,
            tc.tile_pool(name="small", bufs=2) as smallp,
            tc.tile_pool(name="obuf", bufs=4) as obufp,
            tc.tile_pool(name="ps_tp", bufs=2, space="PSUM") as ps_tp,
            tc.tile_pool(name="ps_mm", bufs=4, space="PSUM") as ps_mm,
            tc.tile_pool(name="ps_acc", bufs=2, space="PSUM") as ps_acc,
        ):
            # ---- constants ----
            ident = consts.tile([128, 128], F32, tag="ident")
            nc.gpsimd.memset(ident[:], 0.0)
            nc.gpsimd.affine_select(
                out=ident[:], in_=ident[:],
                compare_op=mybir.AluOpType.not_equal, fill=1.0, base=0,
                pattern=[[-1, 128]], channel_multiplier=1,
            )
            # rounded copy for the f32r transposes (0.0/1.0 are exact in
            # any rounding); a separate tile keeps the f32 producers out of
            # the verifier's f32r producer chain
            identr = consts.tile([128, 128], F32, tag="identr")
            nc.vector.tensor_copy(r32(identr[:]), ident[:])
            ct_sb = consts.tile([128, 8, NCONV], F32, tag="ct")
            # issue from the ACT sequencer: SP's serial ~1us/DMA descriptor
            # chain for the batch-0 ctx loads stays one entry shorter
            nc.scalar.dma_start(out=r32(ct_sb[:]), in_=r32(ct.rearrange("(j p) k -> p j k", p=128)))
            wq_sb = consts.tile([128, 3, U], F32, tag="wq")
            wk_sb = consts.tile([128, 3, U], F32, tag="wk")
            bq_sb = consts.tile([128, 2], F32, tag="bq")
            bk_sb = consts.tile([128, 2], F32, tag="bk")
            kt_sb = consts.tile([128, 9, M], F32, tag="kt")

            import contextlib
            loop_cm = tc.For_i(0, repeat, 1) if repeat else contextlib.nullcontext()
            with loop_cm:
              for b in range(n_batch):
                  # ---- load context: mem rows 0..1023 = ctx, 1024..1037 = conv ----
                  mem_t = memp.tile([128, 9, F], F32, tag="mem")
                  xsr = xs[b].rearrange("(j p) f -> p j f", p=128)
                  xar = xa[b].rearrange("(j p) f -> p j f", p=128)
                  xrr = xr[b].rearrange("(j p) f -> p j f", p=128)
                  # two j-groups per tensor: conv/transposes for tiles 0..3
                  # can start as soon as the first half arrives
                  for g0, g1 in ((0, 4), (4, 8)):
                      nc.sync.dma_start(
                          out=r32(mem_t[:, g0:g1, 0:DS]),
                          in_=r32(xsr[:, g0:g1, :]),
                      )
                      nc.sync.dma_start(
                          out=r32(mem_t[:, g0:g1, DS:DS + DA]),
                          in_=r32(xar[:, g0:g1, :]),
                      )
                      nc.sync.dma_start(
                          out=r32(mem_t[:, g0:g1, DS + DA:F]),
                          in_=r32(xrr[:, g0:g1, :]),
                      )

                  if b == 0:
                      # projection weights are first needed by qkT; emitting
                      # them after the ctx DMAs keeps the conv start unblocked
                      nc.sync.dma_start(
                          out=r32(wq_sb[:]),
                          in_=r32(wq.rearrange("(t p) u -> p t u", p=128)),
                      )
                      nc.sync.dma_start(
                          out=r32(wk_sb[:]),
                          in_=r32(wk.rearrange("(t p) u -> p t u", p=128)),
                      )
                      nc.sync.dma_start(
                          out=bq_sb[:], in_=bq.rearrange("(t p) -> p t", p=128)
                      )
                      nc.sync.dma_start(
                          out=bk_sb[:], in_=bk.rearrange("(t p) -> p t", p=128)
                      )

                  # ---- conv rows: [14, F] = C @ ctx ----
                  conv_ps = ps_tp.tile([128, 512], F32, tag="tp")
                  for j in range(8):
                      nc.tensor.matmul(
                          conv_ps[:NCONV, :F],
                          lhsT=r32(ct_sb[:, j, :]),
                          rhs=r32(mem_t[:, j, :]),
                          start=(j == 0),
                          stop=(j == 7),
                      )
                  nc.scalar.copy(out=r32(mem_t[:NCONV, 8, :]), in_=conv_ps[:NCONV, :F])

                  # ---- memT via PE transposes ----
                  # 4 transposes share one [128,512] psum bank; one wide
                  # psum->sbuf copy per group keeps the copy count low, and the
                  # qkT matmuls for chunk c are emitted right after the groups
                  # they need so PE always has matmul work while copies drain
                  memT_t = memtp.tile([128, 3, M], F32, tag="memT")
                  qT = qkp.tile([128, 2, M], F32, tag="qT")
                  kT = qkp.tile([128, 2, M], F32, tag="kT")
                  JG = [(0, [0, 1, 2, 3]), (1, [4, 5, 6, 7]), (2, [8])]
                  for ci, js in JG:
                      m0, mw = MCH[ci]
                      for ft in range(3):
                          tg_ps = ps_tp.tile([128, 512], F32, tag="tp")
                          for gi, j in enumerate(js):
                              pp = JT[j][1]
                              nc.tensor.transpose(
                                  r32(tg_ps[:, gi * 128:gi * 128 + pp]),
                                  in_=r32(mem_t[:pp, j, ft * 128:(ft + 1) * 128]),
                                  identity=r32(identr[:pp, :pp]),
                              )
                          if ft % 2 == 0:
                              nc.vector.tensor_copy(
                                  r32(memT_t[:, ft, m0:m0 + mw]), tg_ps[:, :mw]
                              )
                          else:
                              nc.scalar.copy(
                                  out=r32(memT_t[:, ft, m0:m0 + mw]),
                                  in_=tg_ps[:, :mw],
                              )
                      for wsb, bsb, dst in ((wq_sb, bq_sb, qT), (wk_sb, bk_sb, kT)):
                          for ut in range(2):
                              ps = ps_mm.tile([128, 512], F32, tag="mm")
                              for ft in range(3):
                                  nc.tensor.matmul(
                                      ps[:, :mw],
                                      lhsT=r32(wsb[:, ft, ut * 128:(ut + 1) * 128]),
                                      rhs=r32(memT_t[:, ft, m0:m0 + mw]),
                                      start=(ft == 0),
                                      stop=(ft == 2),
                                  )
                              nc.scalar.activation(
                                  out=r32(dst[:, ut, m0:m0 + mw]),
                                  in_=ps[:, :mw],
                                  func=mybir.ActivationFunctionType.Identity,
                                  bias=bsb[:, ut:ut + 1],
                              )

                  if b == 0:
                      # the mixing kernel is first needed by the kernel-mixing
                      # phase; loading it here keeps batch-0 ctx DMAs unblocked
                      ktr = kt[0:1024, :].rearrange("(j p) i -> p j i", p=128)
                      nc.sync.dma_start(
                          out=r32(kt_sb[:, 0:3, :]), in_=r32(ktr[:, 0:3, :])
                      )
                      nc.sync.dma_start(
                          out=r32(kt_sb[:, 3:6, :]), in_=r32(ktr[:, 3:6, :])
                      )
                      nc.sync.dma_start(
                          out=r32(kt_sb[:, 6:8, :]), in_=r32(ktr[:, 6:8, :])
                      )
                      nc.sync.dma_start(
                          out=r32(kt_sb[:NCONV, 8, :]), in_=r32(kt[1024:M, :])
                      )

                  # ---- S rows -> probs strips + E^T chunks + att ----
                  # E^T m-chunk c is emitted right before the strips whose
                  # att contraction needs it (jm 0..3 -> chunk0, 4..7 ->
                  # chunk1, 8 -> chunk2), spreading the ACT exp load
                  ET_t = etp.tile([128, 9, M], F32, tag="ET")
                  att_t = attp.tile([128, 9, F], F32, tag="att")
                  r_sb = smallp.tile([128, 9], F32, tag="r")
                  for jm, pm in JT:
                      if jm in (0, 3, 6):
                          m0, mw = SCH[{0: 0, 3: 1, 6: 2}[jm]]
                          for jn, pn in JT:
                              ps = ps_mm.tile([128, 512], F32, tag="mm")
                              for ut in range(2):
                                  nc.tensor.matmul(
                                      ps[:pn, :mw],
                                      lhsT=r32(kT[:, ut, jn * 128:jn * 128 + pn]),
                                      rhs=r32(qT[:, ut, m0:m0 + mw]),
                                      start=(ut == 0),
                                      stop=(ut == 1),
                                  )
                              nc.scalar.activation(
                                  out=r32(ET_t[:pn, jn, m0:m0 + mw]),
                                  in_=ps[:pn, :mw],
                                  func=mybir.ActivationFunctionType.Exp,
                                  scale=SCALE,
                              )
                      es_t = esp.tile([128, M], F32, tag="Es")
                      zsum = smallp.tile([128, 4], F32, tag="z")
                      for ci, (n0, nw) in enumerate(SCH):
                          ps = ps_mm.tile([128, 512], F32, tag="mm")
                          for ut in range(2):
                              nc.tensor.matmul(
                                  ps[:pm, :nw],
                                  lhsT=r32(qT[:, ut, jm * 128:jm * 128 + pm]),
                                  rhs=r32(kT[:, ut, n0:n0 + nw]),
                                  start=(ut == 0),
                                  stop=(ut == 1),
                              )
                          nc.scalar.activation(
                              out=es_t[:pm, n0:n0 + nw],
                              in_=ps[:pm, :nw],
                              func=mybir.ActivationFunctionType.Exp,
                              scale=SCALE,
                              accum_out=zsum[:pm, ci:ci + 1],
                          )
                      ztot = smallp.tile([128, 1], F32, tag="ztot")
                      nc.vector.reduce_sum(
                          out=ztot[:pm, :], in_=zsum[:pm, 0:3], axis=mybir.AxisListType.X
                      )
                      nc.vector.reciprocal(r_sb[:pm, jm:jm + 1], ztot[:pm, :])
                      nc.vector.tensor_scalar_mul(
                          es_t[:pm, :], es_t[:pm, :], r_sb[:pm, jm:jm + 1]
                      )
                      # DMA probs strip (internal cols [ctx;conv] -> ref [conv;ctx])
                      rr = 14 + jm * 128 if jm < 8 else 0
                      nc.sync.dma_start(
                          out=probs[b, rr:rr + pm, NCONV:M], in_=es_t[:pm, 0:1024]
                      )
                      nc.sync.dma_start(
                          out=probs[b, rr:rr + pm, 0:NCONV], in_=es_t[:pm, 1024:M]
                      )

                      # att(jm) interleaved into the strip loop: balances the
                      # ACT exp load across the S window instead of bunching
                      # all att matmuls after it
                      aps = ps_acc.tile([128, F], F32, tag="acc")
                      for jn, pn in JT:
                          nc.tensor.matmul(
                              aps[:pm, :],
                              lhsT=r32(ET_t[:pn, jn, jm * 128:jm * 128 + pm]),
                              rhs=r32(mem_t[:pn, jn, :]),
                              start=(jn == 0),
                              stop=(jn == 8),
                          )
                      nc.vector.tensor_scalar_mul(
                          r32(att_t[:pm, jm, :]), aps[:pm, :], r_sb[:pm, jm:jm + 1]
                      )

                  # ---- out rows: kernel mixing ----
                  for ji, pi in JT:
                      fps = ps_acc.tile([128, F], F32, tag="acc")
                      for jm, pm in JT:
                          nc.tensor.matmul(
                              fps[:pi, :],
                              lhsT=r32(kt_sb[:pm, jm, ji * 128:ji * 128 + pi]),
                              rhs=r32(att_t[:pm, jm, :]),
                              start=(jm == 0),
                              stop=(jm == 8),
                          )
                      ob = obufp.tile([128, F], F32, tag="ob")
                      nc.vector.tensor_copy(ob[:pi, :], fps[:pi, :])
                      rr = 14 + ji * 128 if ji < 8 else 0
                      nc.sync.dma_start(out=out[b, rr:rr + pi, :], in_=ob[:pi, :])

    nc.compile()
    return nc


def _prep_weights(w1, w2, w3, kernel):
    """Host-side prep of replicated weights (conv matrix C^T and permuted
    transposed mixing kernel)."""
    C = np.zeros((NCONV, T), dtype=np.float32)
    for k in range(8):
        C[k, 128 * k:128 * (k + 1)] = w3
    for j in range(4):
        C[8 + j, 256 * j:256 * (j + 1)] = w2
    for j in range(2):
        C[12 + j, 512 * j:512 * (j + 1)] = w1
    ct = np.ascontiguousarray(C.T)
    # internal index m' -> reference index
    perm = np.concatenate([np.arange(NCONV, M), np.arange(NCONV)])
    kp = np.asarray(kernel, dtype=np.float32)[np.ix_(perm, perm)]
    kt = np.ascontiguousarray(kp.T)
    return ct, kt


_NC_CACHE = {}


def kernel(x_state, x_action, x_reward, w1, w2, w3, Wq, bq, Wk, bk, kernel):
    x_state = np.ascontiguousarray(np.asarray(x_state, dtype=np.float32))
    x_action = np.ascontiguousarray(np.asarray(x_action, dtype=np.float32))
    x_reward = np.ascontiguousarray(np.asarray(x_reward, dtype=np.float32))
    w1 = np.asarray(w1, dtype=np.float32)
    w2 = np.asarray(w2, dtype=np.float32)
    w3 = np.asarray(w3, dtype=np.float32)
    Wq = np.ascontiguousarray(np.asarray(Wq, dtype=np.float32))
    Wk = np.ascontiguousarray(np.asarray(Wk, dtype=np.float32))
    bq = np.ascontiguousarray(np.asarray(bq, dtype=np.float32))
    bk = np.ascontiguousarray(np.asarray(bk, dtype=np.float32))

    ct, kt = _prep_weights(w1, w2, w3, kernel)

    if "nc" not in _NC_CACHE:
        _NC_CACHE["nc"] = build_program(BL)
    nc = _NC_CACHE["nc"]

    in_maps = []
    for c in range(N_CORES):
        sl = slice(c * BL, (c + 1) * BL)
        in_maps.append(
            {
                "xs": x_state[sl],
                "xa": x_action[sl],
                "xr": x_reward[sl],
                "ct": ct,
                "wq": Wq,
                "bq": bq,
                "wk": Wk,
                "bk": bk,
                "kt": kt,
            }
        )

    res = run_bass_kernel_spmd(nc, in_maps, list(range(N_CORES)))
    out = np.concatenate([res.results[c]["out"] for c in range(N_CORES)], axis=0)
    probs = np.concatenate([res.results[c]["probs"] for c in range(N_CORES)], axis=0)
    return out, probs



# revision 36
# speedup vs baseline: 1.2499x; 1.0054x over previous
"""Trainium2 Bass kernel for nn_ContextGenerator (scatter_memory).

Data-parallel over the batch axis: 64 batches -> 8 NeuronCores x 8 batches.
All weights (conv vectors, attention projections, [M,M] mixing kernel) are
replicated. No collectives.

Internal memory-row order is [context rows 0..1023 ; conv rows 1024..1037]
(the reference order is [conv ; context]); the permutation is folded into the
host-side prep of the mixing kernel and into the output DMA offsets.
"""

import numpy as np

import concourse.bass as bass
import concourse.mybir as mybir
import concourse.tile as tile
from concourse import bacc
from concourse.bass_utils import run_bass_kernel_spmd

# Problem constants (hardcoded per harness contract)
B = 64
T = 1024
DS, DA, DR = 256, 96, 32
F = DS + DA + DR            # 384
U = 256                     # attention units
NCONV = 14                  # 8 + 4 + 2 compressed slots
M = T + NCONV               # 1038
N_CORES = 8
BL = B // N_CORES           # 8 batches per core
SCALE = 1.0 / 16.0          # 1/sqrt(U)

F32 = mybir.dt.float32
F32R = mybir.dt.float32r

# m-axis tiles: 8 full 128-partition tiles + one 14-row tile
JT = [(j, 128 if j < 8 else NCONV) for j in range(9)]
# free-axis chunks of the m/n axis (psum bank = 512 fp32)
MCH = [(0, 512), (512, 512), (1024, NCONV)]
# score-phase chunks: every chunk >= 256 wide so f32r streams at 1 cyc/row
# (widths < 256 pay a 4x penalty); aligned to strip groups 0-2 / 3-5 / 6-8
SCH = [(0, 384), (384, 384), (768, 270)]


def r32(ap):
    return ap.bitcast(F32R)


def build_program(n_batch=BL, repeat=0):
    """Build the per-core Bass program. Same program on all 8 cores (SPMD)."""
    nc = bacc.Bacc(None, target_bir_lowering=False)

    xs = nc.declare_dram_parameter("xs", [n_batch, T, DS], F32, isOutput=False)
    xa = nc.declare_dram_parameter("xa", [n_batch, T, DA], F32, isOutput=False)
    xr = nc.declare_dram_parameter("xr", [n_batch, T, DR], F32, isOutput=False)
    ct = nc.declare_dram_parameter("ct", [T, NCONV], F32, isOutput=False)
    wq = nc.declare_dram_parameter("wq", [F, U], F32, isOutput=False)
    bq = nc.declare_dram_parameter("bq", [U], F32, isOutput=False)
    wk = nc.declare_dram_parameter("wk", [F, U], F32, isOutput=False)
    bk = nc.declare_dram_parameter("bk", [U], F32, isOutput=False)
    kt = nc.declare_dram_parameter("kt", [M, M], F32, isOutput=False)
    out = nc.declare_dram_parameter("out", [n_batch, M, F], F32, isOutput=True)
    probs = nc.declare_dram_parameter("probs", [n_batch, M, M], F32, isOutput=True)

    with tile.TileContext(nc) as tc:
        with (
            tc.tile_pool(name="consts", bufs=1) as consts,
            tc.tile_pool(name="mem", bufs=3) as memp,
            tc.tile_pool(name="memT", bufs=1) as memtp,
            tc.tile_pool(name="qk", bufs=1) as qkp,
            tc.tile_pool(name="ET", bufs=1) as etp,
            tc.tile_pool(name="Es", bufs=4) as esp,
            tc.tile_pool(name="att", bufs=2) as attp,
            tc.tile_pool(name="small", bufs=2) as smallp,
            tc.tile_pool(name="obuf", bufs=4) as obufp,
            tc.tile_pool(name="ps_tp", bufs=3, space="PSUM") as ps_tp,
            tc.tile_pool(name="ps_mm", bufs=3, space="PSUM") as ps_mm,
            tc.tile_pool(name="ps_acc", bufs=2, space="PSUM") as ps_acc,
        ):
            # ---- constants ----
            ident = consts.tile([128, 128], F32, tag="ident")
            nc.gpsimd.memset(ident[:], 0.0)
            nc.gpsimd.affine_select(
                out=ident[:], in_=ident[:],
                compare_op=mybir.AluOpType.not_equal, fill=1.0, base=0,
                pattern=[[-1, 128]], channel_multiplier=1,
            )
            # rounded copy for the f32r transposes (0.0/1.0 are exact in
            # any rounding); a separate tile keeps the f32 producers out of
            # the verifier's f32r producer chain
            identr = consts.tile([128, 128], F32, tag="identr")
            nc.vector.tensor_copy(r32(identr[:]), ident[:])
            ct_sb = consts.tile([128, 8, NCONV], F32, tag="ct")
            # issue from the ACT sequencer: SP's serial ~1us/DMA descriptor
            # chain for the batch-0 ctx loads stays one entry shorter
            nc.scalar.dma_start(out=r32(ct_sb[:]), in_=r32(ct.rearrange("(j p) k -> p j k", p=128)))
            wq_sb = consts.tile([128, 3, U], F32, tag="wq")
            wk_sb = consts.tile([128, 3, U], F32, tag="wk")
            bq_sb = consts.tile([128, 2], F32, tag="bq")
            bk_sb = consts.tile([128, 2], F32, tag="bk")
            kt_sb = consts.tile([128, 9, M], F32, tag="kt")

            import contextlib
            loop_cm = tc.For_i(0, repeat, 1) if repeat else contextlib.nullcontext()
            with loop_cm:
              for b in range(n_batch):
                  # ---- load context: mem rows 0..1023 = ctx, 1024..1037 = conv ----
                  mem_t = memp.tile([128, 9, F], F32, tag="mem")
                  xsr = xs[b].rearrange("(j p) f -> p j f", p=128)
                  xar = xa[b].rearrange("(j p) f -> p j f", p=128)
                  xrr = xr[b].rearrange("(j p) f -> p j f", p=128)
                  # two j-groups per tensor: conv/transposes for tiles 0..3
                  # can start as soon as the first half arrives
                  for g0, g1 in ((0, 4), (4, 8)):
                      nc.sync.dma_start(
                          out=r32(mem_t[:, g0:g1, 0:DS]),
                          in_=r32(xsr[:, g0:g1, :]),
                      )
                      nc.sync.dma_start(
                          out=r32(mem_t[:, g0:g1, DS:DS + DA]),
                          in_=r32(xar[:, g0:g1, :]),
                      )
                      nc.sync.dma_start(
                          out=r32(mem_t[:, g0:g1, DS + DA:F]),
                          in_=r32(xrr[:, g0:g1, :]),
                      )

                  if b == 0:
                      # projection weights are first needed by qkT; emitting
                      # them after the ctx DMAs keeps the conv start unblocked
                      nc.sync.dma_start(
                          out=r32(wq_sb[:]),
                          in_=r32(wq.rearrange("(t p) u -> p t u", p=128)),
                      )
                      nc.sync.dma_start(
                          out=r32(wk_sb[:]),
                          in_=r32(wk.rearrange("(t p) u -> p t u", p=128)),
                      )
                      nc.sync.dma_start(
                          out=bq_sb[:], in_=bq.rearrange("(t p) -> p t", p=128)
                      )
                      nc.sync.dma_start(
                          out=bk_sb[:], in_=bk.rearrange("(t p) -> p t", p=128)
                      )

                  # ---- conv rows: [14, F] = C @ ctx ----
                  conv_ps = ps_tp.tile([128, 512], F32, tag="tp")
                  for j in range(8):
                      nc.tensor.matmul(
                          conv_ps[:NCONV, :F],
                          lhsT=r32(ct_sb[:, j, :]),
                          rhs=r32(mem_t[:, j, :]),
                          start=(j == 0),
                          stop=(j == 7),
                      )
                  nc.scalar.copy(out=r32(mem_t[:NCONV, 8, :]), in_=conv_ps[:NCONV, :F])

                  # ---- memT via PE transposes ----
                  # 4 transposes share one [128,512] psum bank; one wide
                  # psum->sbuf copy per group keeps the copy count low, and the
                  # qkT matmuls for chunk c are emitted right after the groups
                  # they need so PE always has matmul work while copies drain
                  memT_t = memtp.tile([128, 3, M], F32, tag="memT")
                  qT = qkp.tile([128, 2, M], F32, tag="qT")
                  kT = qkp.tile([128, 2, M], F32, tag="kT")
                  JG = [(0, [0, 1, 2, 3]), (1, [4, 5, 6, 7]), (2, [8])]
                  for ci, js in JG:
                      m0, mw = MCH[ci]          # transpose-group range (tiles)
                      qm0, qmw = SCH[ci]        # qkT range (>=256, full f32r rate)
                      for ft in range(3):
                          tg_ps = ps_tp.tile([128, 512], F32, tag="tp")
                          for gi, j in enumerate(js):
                              pp = JT[j][1]
                              nc.tensor.transpose(
                                  r32(tg_ps[:, gi * 128:gi * 128 + pp]),
                                  in_=r32(mem_t[:pp, j, ft * 128:(ft + 1) * 128]),
                                  identity=r32(identr[:pp, :pp]),
                              )
                          if ft % 2 == 0:
                              nc.vector.tensor_copy(
                                  r32(memT_t[:, ft, m0:m0 + mw]), tg_ps[:, :mw]
                              )
                          else:
                              nc.scalar.copy(
                                  out=r32(memT_t[:, ft, m0:m0 + mw]),
                                  in_=tg_ps[:, :mw],
                              )
                      for wsb, bsb, dst in ((wq_sb, bq_sb, qT), (wk_sb, bk_sb, kT)):
                          for ut in range(2):
                              ps = ps_mm.tile([128, 512], F32, tag="mm")
                              for ft in range(3):
                                  nc.tensor.matmul(
                                      ps[:, :qmw],
                                      lhsT=r32(wsb[:, ft, ut * 128:(ut + 1) * 128]),
                                      rhs=r32(memT_t[:, ft, qm0:qm0 + qmw]),
                                      start=(ft == 0),
                                      stop=(ft == 2),
                                  )
                              nc.scalar.activation(
                                  out=r32(dst[:, ut, qm0:qm0 + qmw]),
                                  in_=ps[:, :qmw],
                                  func=mybir.ActivationFunctionType.Identity,
                                  bias=bsb[:, ut:ut + 1],
                              )

                  if b == 0:
                      # the mixing kernel is first needed by the kernel-mixing
                      # phase; loading it here keeps batch-0 ctx DMAs unblocked
                      ktr = kt[0:1024, :].rearrange("(j p) i -> p j i", p=128)
                      nc.sync.dma_start(
                          out=r32(kt_sb[:, 0:3, :]), in_=r32(ktr[:, 0:3, :])
                      )
                      nc.sync.dma_start(
                          out=r32(kt_sb[:, 3:6, :]), in_=r32(ktr[:, 3:6, :])
                      )
                      nc.sync.dma_start(
                          out=r32(kt_sb[:, 6:8, :]), in_=r32(ktr[:, 6:8, :])
                      )
                      nc.sync.dma_start(
                          out=r32(kt_sb[:NCONV, 8, :]), in_=r32(kt[1024:M, :])
                      )

                  # ---- S rows -> probs strips + E^T chunks + att ----
                  # E^T m-chunk c is emitted right before the strips whose
                  # att contraction needs it (jm 0..3 -> chunk0, 4..7 ->
                  # chunk1, 8 -> chunk2), spreading the ACT exp load
                  ET_t = etp.tile([128, 9, M], F32, tag="ET")
                  att_t = attp.tile([128, 9, F], F32, tag="att")
                  r_sb = smallp.tile([128, 9], F32, tag="r")
                  for jm, pm in JT:
                      if jm in (0, 3, 6):
                          m0, mw = SCH[{0: 0, 3: 1, 6: 2}[jm]]
                          for jn, pn in JT:
                              ps = ps_mm.tile([128, 512], F32, tag="mm")
                              for ut in range(2):
                                  nc.tensor.matmul(
                                      ps[:pn, :mw],
                                      lhsT=r32(kT[:, ut, jn * 128:jn * 128 + pn]),
                                      rhs=r32(qT[:, ut, m0:m0 + mw]),
                                      start=(ut == 0),
                                      stop=(ut == 1),
                                  )
                              nc.scalar.activation(
                                  out=r32(ET_t[:pn, jn, m0:m0 + mw]),
                                  in_=ps[:pn, :mw],
                                  func=mybir.ActivationFunctionType.Exp,
                                  scale=SCALE,
                              )
                      es_t = esp.tile([128, M], F32, tag="Es")
                      zsum = smallp.tile([128, 4], F32, tag="z")
                      for ci, (n0, nw) in enumerate(SCH):
                          ps = ps_mm.tile([128, 512], F32, tag="mm")
                          for ut in range(2):
                              nc.tensor.matmul(
                                  ps[:pm, :nw],
                                  lhsT=r32(qT[:, ut, jm * 128:jm * 128 + pm]),
                                  rhs=r32(kT[:, ut, n0:n0 + nw]),
                                  start=(ut == 0),
                                  stop=(ut == 1),
                              )
                          nc.scalar.activation(
                              out=es_t[:pm, n0:n0 + nw],
                              in_=ps[:pm, :nw],
                              func=mybir.ActivationFunctionType.Exp,
                              scale=SCALE,
                              accum_out=zsum[:pm, ci:ci + 1],
                          )
                      ztot = smallp.tile([128, 1], F32, tag="ztot")
                      nc.vector.reduce_sum(
                          out=ztot[:pm, :], in_=zsum[:pm, 0:3], axis=mybir.AxisListType.X
                      )
                      nc.vector.reciprocal(r_sb[:pm, jm:jm + 1], ztot[:pm, :])
                      nc.vector.tensor_scalar_mul(
                          es_t[:pm, :], es_t[:pm, :], r_sb[:pm, jm:jm + 1]
                      )
                      # DMA probs strip (internal cols [ctx;conv] -> ref [conv;ctx])
                      rr = 14 + jm * 128 if jm < 8 else 0
                      nc.sync.dma_start(
                          out=probs[b, rr:rr + pm, NCONV:M], in_=es_t[:pm, 0:1024]
                      )
                      nc.sync.dma_start(
                          out=probs[b, rr:rr + pm, 0:NCONV], in_=es_t[:pm, 1024:M]
                      )

                      # att(jm) interleaved into the strip loop: balances the
                      # ACT exp load across the S window instead of bunching
                      # all att matmuls after it
                      aps = ps_acc.tile([128, F], F32, tag="acc")
                      for jn, pn in JT:
                          nc.tensor.matmul(
                              aps[:pm, :],
                              lhsT=r32(ET_t[:pn, jn, jm * 128:jm * 128 + pm]),
                              rhs=r32(mem_t[:pn, jn, :]),
                              start=(jn == 0),
                              stop=(jn == 8),
                          )
                      nc.vector.tensor_scalar_mul(
                          r32(att_t[:pm, jm, :]), aps[:pm, :], r_sb[:pm, jm:jm + 1]
                      )

                  # ---- out rows: kernel mixing ----
                  for ji, pi in JT:
                      fps = ps_acc.tile([128, F], F32, tag="acc")
                      for jm, pm in JT:
                          nc.tensor.matmul(
                              fps[:pi, :],
                              lhsT=r32(kt_sb[:pm, jm, ji * 128:ji * 128 + pi]),
                              rhs=r32(att_t[:pm, jm, :]),
                              start=(jm == 0),
                              stop=(jm == 8),
                          )
                      ob = obufp.tile([128, F], F32, tag="ob")
                      nc.vector.tensor_copy(ob[:pi, :], fps[:pi, :])
                      rr = 14 + ji * 128 if ji < 8 else 0
                      nc.sync.dma_start(out=out[b, rr:rr + pi, :], in_=ob[:pi, :])

    nc.compile()
    return nc


def _prep_weights(w1, w2, w3, kernel):
    """Host-side prep of replicated weights (conv matrix C^T and permuted
    transposed mixing kernel)."""
    C = np.zeros((NCONV, T), dtype=np.float32)
    for k in range(8):
        C[k, 128 * k:128 * (k + 1)] = w3
    for j in range(4):
        C[8 + j, 256 * j:256 * (j + 1)] = w2
    for j in range(2):
        C[12 + j, 512 * j:512 * (j + 1)] = w1
    ct = np.ascontiguousarray(C.T)
    # internal index m' -> reference index
    perm = np.concatenate([np.arange(NCONV, M), np.arange(NCONV)])
    kp = np.asarray(kernel, dtype=np.float32)[np.ix_(perm, perm)]
    kt = np.ascontiguousarray(kp.T)
    return ct, kt


_NC_CACHE = {}


def kernel(x_state, x_action, x_reward, w1, w2, w3, Wq, bq, Wk, bk, kernel):
    x_state = np.ascontiguousarray(np.asarray(x_state, dtype=np.float32))
    x_action = np.ascontiguousarray(np.asarray(x_action, dtype=np.float32))
    x_reward = np.ascontiguousarray(np.asarray(x_reward, dtype=np.float32))
    w1 = np.asarray(w1, dtype=np.float32)
    w2 = np.asarray(w2, dtype=np.float32)
    w3 = np.asarray(w3, dtype=np.float32)
    Wq = np.ascontiguousarray(np.asarray(Wq, dtype=np.float32))
    Wk = np.ascontiguousarray(np.asarray(Wk, dtype=np.float32))
    bq = np.ascontiguousarray(np.asarray(bq, dtype=np.float32))
    bk = np.ascontiguousarray(np.asarray(bk, dtype=np.float32))

    ct, kt = _prep_weights(w1, w2, w3, kernel)

    if "nc" not in _NC_CACHE:
        _NC_CACHE["nc"] = build_program(BL)
    nc = _NC_CACHE["nc"]

    in_maps = []
    for c in range(N_CORES):
        sl = slice(c * BL, (c + 1) * BL)
        in_maps.append(
            {
                "xs": x_state[sl],
                "xa": x_action[sl],
                "xr": x_reward[sl],
                "ct": ct,
                "wq": Wq,
                "bq": bq,
                "wk": Wk,
                "bk": bk,
                "kt": kt,
            }
        )

    res = run_bass_kernel_spmd(nc, in_maps, list(range(N_CORES)))
    out = np.concatenate([res.results[c]["out"] for c in range(N_CORES)], axis=0)
    probs = np.concatenate([res.results[c]["probs"] for c in range(N_CORES)], axis=0)
    return out, probs

